# revision 16
# baseline (speedup 1.0000x reference)
"""AttentionBlock (GroupNorm + 8-head self-attention + proj + residual) on 8 TRN2 cores.

Sharding: data-parallel over batch (8 batch elements -> 8 cores). Each core runs the
full block for one [512, 32*32] image in a single Bass/Tile kernel.

Per-core pipeline (matmul operands in bf16, fp32 accumulation, fp32 elsewhere):
  GroupNorm:   bn_stats/bn_aggr per channel -> group reduce via matmul with a
               0/1 selection matrix -> rsqrt -> broadcast back via matmul ->
               fused scale+bias apply (DVE).
  QKV:         h @ Wqkv^T. Q,K produced as [d, s] zero-padded to 128 partitions
               per head (so attention matmuls run K=128, full array mode);
               V produced transposed ([s, d]) by swapping matmul operands,
               so attention needs no transposes.
  Attention:   S^T[b,a] = K^T Q per head, P^T = exp(S^T/8) on ACT in [128,1024]
               slabs (softmax max-subtraction skipped: logits are O(1) by
               construction), AV via [v^T | 1x8] augmented weights -> psum rows
               0-63 = unnormalized out, rows 64-71 = Z; normalize with a
               multi-lane reciprocal + GPSIMD partition-broadcast.
  Proj+res:    att @ Wproj^T + proj_b + x.
"""
import sys

sys.path.insert(0, "/opt/trn_rl_repo")

import numpy as np
import ml_dtypes

import concourse.bass as bass
import concourse.bacc as bacc
import concourse.tile as tile
from concourse import mybir
from concourse.bass_utils import run_bass_kernel_spmd

F32 = mybir.dt.float32
BF16 = mybir.dt.bfloat16
AF = mybir.ActivationFunctionType
OP = mybir.AluOpType
NPBF16 = ml_dtypes.bfloat16

P = 128
CT = 4  # channel tiles (512 / 128)
S = 1024  # spatial positions (32*32)
HEADS = 8
D = 64
M_AV = D + 8  # AV stationary cols: 64 v + 8 ones (Z lands on psum rows 64-71)
N_CORES = 8
EPS = 1e-5


def _emit(nc, tc, ctx):
    x_d = nc.dram_tensor("x", [512, S], F32, kind="ExternalInput")
    wqkv_d = nc.dram_tensor("wqkv", [P, CT, 1536], BF16, kind="ExternalInput")
    wproj_d = nc.dram_tensor("wproj", [P, CT, 512], BF16, kind="ExternalInput")
    gnw_d = nc.dram_tensor("gnw", [P, CT], F32, kind="ExternalInput")
    gnb_d = nc.dram_tensor("gnb", [P, CT], F32, kind="ExternalInput")
    qkvb_d = nc.dram_tensor("qkvb", [P, 8], F32, kind="ExternalInput")
    vb_d = nc.dram_tensor("vb", [512], F32, kind="ExternalInput")
    projb_d = nc.dram_tensor("projb", [P, CT], F32, kind="ExternalInput")
    sel_d = nc.dram_tensor("sel", [P, P], BF16, kind="ExternalInput")
    selt_d = nc.dram_tensor("selt", [P, P], BF16, kind="ExternalInput")
    out_d = nc.dram_tensor("out", [512, S], F32, kind="ExternalOutput")

    consts = ctx.enter_context(tc.tile_pool(name="consts", bufs=1))
    big = ctx.enter_context(tc.tile_pool(name="big", bufs=1))
    small = ctx.enter_context(tc.tile_pool(name="small", bufs=2))
    ptp = ctx.enter_context(tc.tile_pool(name="ptp", bufs=2))
    outp = ctx.enter_context(tc.tile_pool(name="outp", bufs=3))
    ps = ctx.enter_context(tc.tile_pool(name="ps", bufs=2, space="PSUM"))
    psav = ctx.enter_context(tc.tile_pool(name="psav", bufs=2, space="PSUM"))

    # ---- input DMAs (x first: GroupNorm needs it immediately) ----
    x_all = big.tile([P, CT, S], F32)
    xv = x_d[:, :].rearrange("(j p) s -> p j s", p=P)
    for j in range(CT):
        nc.sync.dma_start(out=x_all[:, j, :], in_=xv[:, j, :])
    gnw = consts.tile([P, CT], F32)
    nc.sync.dma_start(out=gnw, in_=gnw_d[:, :])
    gnb = consts.tile([P, CT], F32)
    nc.sync.dma_start(out=gnb, in_=gnb_d[:, :])
    sel = consts.tile([P, P], BF16)
    nc.sync.dma_start(out=sel, in_=sel_d[:, :])
    selt = consts.tile([P, P], BF16)
    nc.sync.dma_start(out=selt, in_=selt_d[:, :])
    wqkv = consts.tile([P, CT, 1536], BF16)
    nc.sync.dma_start(out=wqkv, in_=wqkv_d[:, :, :])
    qkvb = consts.tile([P, 8], F32)
    nc.sync.dma_start(out=qkvb, in_=qkvb_d[:, :])
    projb = consts.tile([P, CT], F32)
    nc.sync.dma_start(out=projb, in_=projb_d[:, :])
    vb = consts.tile([P, 512], F32)
    vb_ap = vb_d[:]
    vb_bcast = bass.AP(tensor=vb_ap.tensor, offset=vb_ap.offset, ap=[[0, P], vb_ap.ap[0]])
    nc.gpsimd.dma_start(out=vb, in_=vb_bcast)
    wproj = consts.tile([P, CT, 512], BF16)
    nc.sync.dma_start(out=wproj, in_=wproj_d[:, :, :])

    eps_t = consts.tile([P, 1], F32)
    nc.vector.memset(eps_t, EPS)
    zeros8 = consts.tile([P, 8], F32)
    nc.vector.memset(zeros8, 0.0)
    ones64 = consts.tile([P, 64], BF16)
    nc.vector.memset(ones64, 1.0)

    h_all = big.tile([P, CT, S], BF16)
    # q: 4 packed head-pair tiles (0-3). k: 8 per-head tiles (4-11) zero-padded
    # to 128 partitions -- head h's 64 d-rows sit at partitions (h%2)*64, the
    # other half stays zero so attention matmuls run K=128 in full-array mode.
    qk = big.tile([P, 12, S], BF16)
    nc.vector.memset(qk[:, 4:12, :], 0.0)

    vt = big.tile([P, 8, HEADS, M_AV], BF16)
    att = big.tile([P, CT, S], BF16)

    # ---- GroupNorm statistics ----
    stats = small.tile([P, CT, 2, 6], F32)
    mv = small.tile([P, CT, 2], F32)
    for j in range(CT):
        for sg in range(2):
            nc.vector.bn_stats(out=stats[:, j, sg, :], in_=x_all[:, j, sg * 512:(sg + 1) * 512])
        nc.vector.bn_aggr(out=mv[:, j, :], in_=stats[:, j, :, :])
    means = mv[:, :, 0]
    vars_ = mv[:, :, 1]
    stats2 = small.tile([P, 8], F32)
    nc.vector.tensor_copy(out=stats2[:, 0:4], in_=means)
    nc.vector.tensor_mul(out=stats2[:, 4:8], in0=means, in1=means)
    nc.vector.tensor_add(out=stats2[:, 4:8], in0=stats2[:, 4:8], in1=vars_)
    statsr = small.tile([P, 8], BF16)
    nc.vector.tensor_copy(out=statsr, in_=stats2)

    psum_g = ps.tile([P, 8], F32, tag="mm2")
    nc.tensor.matmul(psum_g[:, :], lhsT=sel[:, :], rhs=statsr[:, :], start=True, stop=True)

    tmv = small.tile([P, 8], F32)
    nc.vector.tensor_scalar_mul(out=tmv[0:8, :], in0=psum_g[0:8, :], scalar1=1.0 / 16.0)
    gm = tmv[0:8, 0:4]
    gm2 = tmv[0:8, 4:8]
    var_t = small.tile([P, 4], F32)
    nc.vector.tensor_mul(out=var_t[0:8, :], in0=gm, in1=gm)
    nc.vector.tensor_sub(out=var_t[0:8, :], in0=gm2, in1=var_t[0:8, :])
    nc.scalar.activation(out=var_t[0:8, :], in_=var_t[0:8, :], func=AF.Sqrt, bias=eps_t[0:8, :], scale=1.0)
    a_t = small.tile([P, 4], F32)
    nc.vector.reciprocal(out=a_t[0:8, :], in_=var_t[0:8, :])
    b_t = small.tile([P, 4], F32)
    nc.vector.tensor_mul(out=b_t[0:8, :], in0=gm, in1=a_t[0:8, :])
    abr = small.tile([P, 8], BF16)
    nc.vector.tensor_copy(out=abr, in_=zeros8)
    nc.vector.tensor_copy(out=abr[0:8, 0:4], in_=a_t[0:8, :])
    nc.vector.tensor_scalar(out=abr[0:8, 4:8], in0=b_t[0:8, :], scalar1=-1.0, scalar2=None, op0=OP.mult)

    # GroupNorm group stats in bf16 would lose ~0.4%; the scale/bias path keeps
    # everything fp32 except the two tiny matmul hops (sel/selt are exact 0/1).
    psum_ab = ps.tile([P, 8], F32, tag="mm2")
    nc.tensor.matmul(psum_ab[:, :], lhsT=selt[:, :], rhs=abr[:, :], start=True, stop=True)

    scb = small.tile([P, CT, 2], F32)
    for j in range(CT):
        nc.vector.tensor_mul(out=scb[:, j, 0:1], in0=psum_ab[:, j:j + 1], in1=gnw[:, j:j + 1])
        nc.vector.tensor_mul(out=scb[:, j, 1:2], in0=psum_ab[:, 4 + j:5 + j], in1=gnw[:, j:j + 1])
        nc.vector.tensor_add(out=scb[:, j, 1:2], in0=scb[:, j, 1:2], in1=gnb[:, j:j + 1])
        nc.vector.tensor_scalar(
            out=h_all[:, j, :], in0=x_all[:, j, :],
            scalar1=scb[:, j, 0:1], scalar2=scb[:, j, 1:2],
            op0=OP.mult, op1=OP.add,
        )

    # ---- QKV + attention, interleaved per head-pair ----
    # V first, then per pair p: Q/K projections for p, then p's attention.
    # This hands ACT its exp work as early as possible instead of letting it
    # idle through the whole QKV phase.
    for si in range(8):  # V in [s, d] orientation (transposed for free)
        pv = ps.tile([P, 512], F32, tag="mm2")
        for kc in range(CT):
            nc.tensor.matmul(
                pv[:, :],
                lhsT=h_all[:, kc, si * 128:(si + 1) * 128],
                rhs=wqkv[:, kc, 1024:1536],
                start=(kc == 0), stop=(kc == CT - 1),
            )
        nc.vector.tensor_add(
            out=vt[:, si, :, 0:D],
            in0=pv[:, :].rearrange("p (h d) -> p h d", h=HEADS),
            in1=vb[:, :].rearrange("p (h d) -> p h d", h=HEADS),
        )
        nc.vector.tensor_copy(
            out=vt[:, si, :, D:M_AV],
            in_=ones64[:, :].rearrange("p (h o) -> p h o", h=HEADS),
        )

    for pc in range(4):  # head pairs
        for oi in (pc, pc + 4):  # Q tile then K tile for this pair
            pq = ps.tile([P, S], F32, tag="mm2")
            for kc in range(CT):
                for nh in range(2):
                    nc.tensor.matmul(
                        pq[:, nh * 512:(nh + 1) * 512],
                        lhsT=wqkv[:, kc, oi * 128:(oi + 1) * 128],
                        rhs=h_all[:, kc, nh * 512:(nh + 1) * 512],
                        start=(kc == 0), stop=(kc == CT - 1),
                    )
            if oi < 4:  # q: packed pair tile, one eviction
                nc.vector.tensor_scalar(
                    out=qk[:, oi, :], in0=pq[:, :],
                    scalar1=qkvb[:, oi:oi + 1], scalar2=None, op0=OP.add,
                )
            else:  # k: split into per-head padded tiles (partitions preserved)
                hd = 2 * (oi - 4)
                nc.vector.tensor_scalar(
                    out=qk[0:64, 4 + hd, :], in0=pq[0:64, :],
                    scalar1=qkvb[0:64, oi:oi + 1], scalar2=None, op0=OP.add,
                )
                nc.vector.tensor_scalar(
                    out=qk[64:P, 4 + hd + 1, :], in0=pq[64:P, :],
                    scalar1=qkvb[64:P, oi:oi + 1], scalar2=None, op0=OP.add,
                )

        pt = ptp.tile([P, 2, 8, S], BF16, tag="pt")
        for bi in range(8):
            for hp in range(2):
                hd = 2 * pc + hp
                pS = ps.tile([P, S], F32, tag="mm2")
                for ah in range(2):
                    nc.tensor.matmul(
                        pS[:, ah * 512:(ah + 1) * 512],
                        lhsT=qk[:, 4 + hd, bi * 128:(bi + 1) * 128],
                        rhs=qk[:, pc, ah * 512:(ah + 1) * 512],
                        start=True, stop=True,
                    )
                nc.scalar.activation(out=pt[:, hp, bi, :], in_=pS[:, :], func=AF.Exp, scale=0.125)
        for hp in range(2):
            hd = 2 * pc + hp
            pav = psav.tile([P, S], F32, tag="av")
            for bi in range(8):
                for ah in range(2):
                    nc.tensor.matmul(
                        pav[0:M_AV, ah * 512:(ah + 1) * 512],
                        lhsT=vt[:, bi, hd, :],
                        rhs=pt[:, hp, bi, ah * 512:(ah + 1) * 512],
                        start=(bi == 0), stop=(bi == 7),
                    )
            zc = small.tile([P, S], F32, tag="zc")
            nc.vector.tensor_copy(out=zc[0:8, :], in_=pav[D:D + 8, :])
            zs = small.tile([P, S], F32, tag="zs")
            nc.vector.reciprocal_approx_fast(out=zs[0:8, :], in_=zc[0:8, :])
            zb = small.tile([P, S], F32, tag="zb")
            nc.gpsimd.partition_broadcast(zb[0:D, :], zs[0:1, :])
            if hp == 0:
                nc.vector.tensor_mul(out=att[0:D, pc, :], in0=pav[0:D, :], in1=zb[0:D, :])
            else:
                stg = small.tile([P, S], BF16, tag="stg")
                nc.vector.tensor_mul(out=stg[0:D, :], in0=pav[0:D, :], in1=zb[0:D, :])
                nc.sync.dma_start(out=att[D:P, pc, :], in_=stg[0:D, :])

    # ---- proj + bias + residual ----
    out_view = out_d[:, :].rearrange("(j p) s -> p j s", p=P)
    for oi in range(CT):
        pp = ps.tile([P, S], F32, tag="mm2")
        for pc in range(4):
            for sh in range(2):
                nc.tensor.matmul(
                    pp[:, sh * 512:(sh + 1) * 512],
                    lhsT=wproj[:, pc, oi * 128:(oi + 1) * 128],
                    rhs=att[:, pc, sh * 512:(sh + 1) * 512],
                    start=(pc == 0), stop=(pc == 3),
                )
        ot = outp.tile([P, S], F32, tag="o")
        nc.vector.tensor_scalar(out=ot, in0=pp[:, :], scalar1=projb[:, oi:oi + 1], scalar2=None, op0=OP.add)
        nc.vector.tensor_add(out=ot, in0=ot, in1=x_all[:, oi, :])
        nc.sync.dma_start(out=out_view[:, oi, :], in_=ot)


_NC_CACHE = None


def _build():
    global _NC_CACHE
    if _NC_CACHE is None:
        from contextlib import ExitStack

        nc = bacc.Bacc()
        with tile.TileContext(nc) as tc:
            with ExitStack() as ctx:
                _emit(nc, tc, ctx)
        nc.finalize()
        _NC_CACHE = nc
    return _NC_CACHE


def _prep_inputs(inputs):
    x = np.ascontiguousarray(np.asarray(inputs["x"], dtype=np.float32))  # [8,512,32,32]
    gn_w = np.asarray(inputs["gn_w"], dtype=np.float32)
    gn_b = np.asarray(inputs["gn_b"], dtype=np.float32)
    qkv_w = np.asarray(inputs["qkv_w"], dtype=np.float32)
    qkv_b = np.asarray(inputs["qkv_b"], dtype=np.float32)
    proj_w = np.asarray(inputs["proj_w"], dtype=np.float32)
    proj_b = np.asarray(inputs["proj_b"], dtype=np.float32)

    wqkv_p = np.ascontiguousarray(
        qkv_w.T.reshape(CT, P, 1536).transpose(1, 0, 2).astype(NPBF16)
    )
    wproj_p = np.ascontiguousarray(
        proj_w.T.reshape(CT, P, 512).transpose(1, 0, 2).astype(NPBF16)
    )
    gnw_p = np.ascontiguousarray(gn_w.reshape(CT, P).T)
    gnb_p = np.ascontiguousarray(gn_b.reshape(CT, P).T)
    qkvb_p = np.ascontiguousarray(qkv_b[:1024].reshape(8, P).T)
    vb_p = np.ascontiguousarray(qkv_b[1024:])
    projb_p = np.ascontiguousarray(proj_b.reshape(CT, P).T)
    sel = np.zeros((P, P), dtype=NPBF16)
    for p in range(P):
        sel[p, p // 16] = 1.0
    selt = np.ascontiguousarray(sel.T)

    shared = {
        "wqkv": wqkv_p, "wproj": wproj_p, "gnw": gnw_p, "gnb": gnb_p,
        "qkvb": qkvb_p, "vb": vb_p, "projb": projb_p, "sel": sel, "selt": selt,
    }
    in_maps = []
    for i in range(N_CORES):
        m = dict(shared)
        m["x"] = np.ascontiguousarray(x[i].reshape(512, S))
        in_maps.append(m)
    return in_maps


def run(inputs, trace=False, tmpdir=None):
    nc = _build()
    in_maps = _prep_inputs(inputs)
    res = run_bass_kernel_spmd(
        nc, in_maps, core_ids=list(range(N_CORES)), trace=trace, tmpdir=tmpdir
    )
    out = np.stack([res.results[i]["out"] for i in range(N_CORES)])
    return out.reshape(N_CORES, 512, 32, 32), res


def kernel(**inputs):
    out, _ = run(inputs, trace=False)
    return out


# revision 17
# speedup vs baseline: 1.2346x; 1.2346x over previous
"""AttentionBlock (GroupNorm + 8-head self-attention + proj + residual) on 8 TRN2 cores.

Sharding: data-parallel over batch (8 batch elements -> 8 cores). Each core runs the
full block for one [512, 32*32] image in a single Bass/Tile kernel.

Per-core pipeline (matmul operands in bf16, fp32 accumulation, fp32 elsewhere):
  GroupNorm:   bn_stats/bn_aggr per channel -> group reduce via matmul with a
               0/1 selection matrix -> rsqrt -> broadcast back via matmul ->
               fused scale+bias apply (DVE).
  QKV:         h @ Wqkv^T. Q,K produced as [d, s] zero-padded to 128 partitions
               per head (so attention matmuls run K=128, full array mode);
               V produced transposed ([s, d]) by swapping matmul operands,
               so attention needs no transposes.
  Attention:   S^T[b,a] = K^T Q per head, P^T = exp(S^T/8) on ACT in [128,1024]
               slabs (softmax max-subtraction skipped: logits are O(1) by
               construction), AV via [v^T | 1x8] augmented weights -> psum rows
               0-63 = unnormalized out, rows 64-71 = Z; normalize with a
               multi-lane reciprocal + GPSIMD partition-broadcast.
  Proj+res:    att @ Wproj^T + proj_b + x.
"""
import sys

sys.path.insert(0, "/opt/trn_rl_repo")

import numpy as np
import ml_dtypes

import concourse.bass as bass
import concourse.bacc as bacc
import concourse.tile as tile
from concourse import mybir
from concourse.bass_utils import run_bass_kernel_spmd

F32 = mybir.dt.float32
BF16 = mybir.dt.bfloat16
AF = mybir.ActivationFunctionType
OP = mybir.AluOpType
NPBF16 = ml_dtypes.bfloat16

P = 128
CT = 4  # channel tiles (512 / 128)
S = 1024  # spatial positions (32*32)
HEADS = 8
D = 64
M_AV = D + 8  # AV stationary cols: 64 v + 8 ones (Z lands on psum rows 64-71)
N_CORES = 8
EPS = 1e-5


def _emit(nc, tc, ctx):
    x_d = nc.dram_tensor("x", [512, S], F32, kind="ExternalInput")
    wqkv_d = nc.dram_tensor("wqkv", [P, CT, 1536], BF16, kind="ExternalInput")
    wproj_d = nc.dram_tensor("wproj", [P, CT, 512], BF16, kind="ExternalInput")
    gnw_d = nc.dram_tensor("gnw", [P, CT], F32, kind="ExternalInput")
    gnb_d = nc.dram_tensor("gnb", [P, CT], F32, kind="ExternalInput")
    qkvb_d = nc.dram_tensor("qkvb", [P, 8], F32, kind="ExternalInput")
    vb_d = nc.dram_tensor("vb", [512], F32, kind="ExternalInput")
    projb_d = nc.dram_tensor("projb", [P, CT], F32, kind="ExternalInput")
    sel_d = nc.dram_tensor("sel", [P, P], BF16, kind="ExternalInput")
    selt_d = nc.dram_tensor("selt", [P, P], BF16, kind="ExternalInput")
    out_d = nc.dram_tensor("out", [512, S], F32, kind="ExternalOutput")

    consts = ctx.enter_context(tc.tile_pool(name="consts", bufs=1))
    big = ctx.enter_context(tc.tile_pool(name="big", bufs=1))
    small = ctx.enter_context(tc.tile_pool(name="small", bufs=2))
    ptp = ctx.enter_context(tc.tile_pool(name="ptp", bufs=2))
    outp = ctx.enter_context(tc.tile_pool(name="outp", bufs=3))
    ps = ctx.enter_context(tc.tile_pool(name="ps", bufs=2, space="PSUM"))
    psav = ctx.enter_context(tc.tile_pool(name="psav", bufs=2, space="PSUM"))

    # ---- input DMAs (x first: GroupNorm needs it immediately) ----
    x_all = big.tile([P, CT, S], F32)
    xv = x_d[:, :].rearrange("(j p) s -> p j s", p=P)
    for j in range(CT):
        nc.sync.dma_start(out=x_all[:, j, :], in_=xv[:, j, :])
    gnw = consts.tile([P, CT], F32)
    nc.sync.dma_start(out=gnw, in_=gnw_d[:, :])
    gnb = consts.tile([P, CT], F32)
    nc.sync.dma_start(out=gnb, in_=gnb_d[:, :])
    sel = consts.tile([P, P], BF16)
    nc.sync.dma_start(out=sel, in_=sel_d[:, :])
    selt = consts.tile([P, P], BF16)
    nc.sync.dma_start(out=selt, in_=selt_d[:, :])
    wqkv = consts.tile([P, CT, 1536], BF16)
    nc.sync.dma_start(out=wqkv, in_=wqkv_d[:, :, :])
    qkvb = consts.tile([P, 8], F32)
    nc.sync.dma_start(out=qkvb, in_=qkvb_d[:, :])
    projb = consts.tile([P, CT], F32)
    nc.sync.dma_start(out=projb, in_=projb_d[:, :])
    vb = consts.tile([P, 512], F32)
    vb_ap = vb_d[:]
    vb_bcast = bass.AP(tensor=vb_ap.tensor, offset=vb_ap.offset, ap=[[0, P], vb_ap.ap[0]])
    nc.gpsimd.dma_start(out=vb, in_=vb_bcast)
    wproj = consts.tile([P, CT, 512], BF16)
    nc.sync.dma_start(out=wproj, in_=wproj_d[:, :, :])

    eps_t = consts.tile([P, 1], F32)
    nc.vector.memset(eps_t, EPS)
    zeros8 = consts.tile([P, 8], F32)
    nc.vector.memset(zeros8, 0.0)
    ones64 = consts.tile([P, 64], BF16)
    nc.vector.memset(ones64, 1.0)

    h_all = big.tile([P, CT, S], BF16)
    # q: 4 packed head-pair tiles (0-3). k: 8 per-head tiles (4-11) zero-padded
    # to 128 partitions -- head h's 64 d-rows sit at partitions (h%2)*64, the
    # other half stays zero so attention matmuls run K=128 in full-array mode.
    qk = big.tile([P, 12, S], BF16)
    nc.vector.memset(qk[:, 4:12, :], 0.0)

    vt = big.tile([P, 8, HEADS, M_AV], BF16)
    att = big.tile([P, CT, S], BF16)

    # ---- GroupNorm statistics ----
    stats = small.tile([P, CT, 2, 6], F32)
    mv = small.tile([P, CT, 2], F32)
    for j in range(CT):
        for sg in range(2):
            nc.vector.bn_stats(out=stats[:, j, sg, :], in_=x_all[:, j, sg * 512:(sg + 1) * 512])
        nc.vector.bn_aggr(out=mv[:, j, :], in_=stats[:, j, :, :])
    means = mv[:, :, 0]
    vars_ = mv[:, :, 1]
    stats2 = small.tile([P, 8], F32)
    nc.vector.tensor_copy(out=stats2[:, 0:4], in_=means)
    nc.vector.tensor_mul(out=stats2[:, 4:8], in0=means, in1=means)
    nc.vector.tensor_add(out=stats2[:, 4:8], in0=stats2[:, 4:8], in1=vars_)
    statsr = small.tile([P, 8], BF16)
    nc.vector.tensor_copy(out=statsr, in_=stats2)

    psum_g = ps.tile([P, 8], F32, tag="mm2")
    nc.tensor.matmul(psum_g[:, :], lhsT=sel[:, :], rhs=statsr[:, :], start=True, stop=True)

    tmv = small.tile([P, 8], F32)
    nc.vector.tensor_scalar_mul(out=tmv[0:8, :], in0=psum_g[0:8, :], scalar1=1.0 / 16.0)
    gm = tmv[0:8, 0:4]
    gm2 = tmv[0:8, 4:8]
    var_t = small.tile([P, 4], F32)
    nc.vector.tensor_mul(out=var_t[0:8, :], in0=gm, in1=gm)
    nc.vector.tensor_sub(out=var_t[0:8, :], in0=gm2, in1=var_t[0:8, :])
    nc.scalar.activation(out=var_t[0:8, :], in_=var_t[0:8, :], func=AF.Sqrt, bias=eps_t[0:8, :], scale=1.0)
    a_t = small.tile([P, 4], F32)
    nc.vector.reciprocal(out=a_t[0:8, :], in_=var_t[0:8, :])
    b_t = small.tile([P, 4], F32)
    nc.vector.tensor_mul(out=b_t[0:8, :], in0=gm, in1=a_t[0:8, :])
    abr = small.tile([P, 8], BF16)
    nc.vector.tensor_copy(out=abr, in_=zeros8)
    nc.vector.tensor_copy(out=abr[0:8, 0:4], in_=a_t[0:8, :])
    nc.vector.tensor_scalar(out=abr[0:8, 4:8], in0=b_t[0:8, :], scalar1=-1.0, scalar2=None, op0=OP.mult)

    # GroupNorm group stats in bf16 would lose ~0.4%; the scale/bias path keeps
    # everything fp32 except the two tiny matmul hops (sel/selt are exact 0/1).
    psum_ab = ps.tile([P, 8], F32, tag="mm2")
    nc.tensor.matmul(psum_ab[:, :], lhsT=selt[:, :], rhs=abr[:, :], start=True, stop=True)

    scb = small.tile([P, CT, 2], F32)
    for j in range(CT):
        nc.vector.tensor_mul(out=scb[:, j, 0:1], in0=psum_ab[:, j:j + 1], in1=gnw[:, j:j + 1])
        nc.vector.tensor_mul(out=scb[:, j, 1:2], in0=psum_ab[:, 4 + j:5 + j], in1=gnw[:, j:j + 1])
        nc.vector.tensor_add(out=scb[:, j, 1:2], in0=scb[:, j, 1:2], in1=gnb[:, j:j + 1])
        nc.vector.tensor_scalar(
            out=h_all[:, j, :], in0=x_all[:, j, :],
            scalar1=scb[:, j, 0:1], scalar2=scb[:, j, 1:2],
            op0=OP.mult, op1=OP.add,
        )

    # ---- QKV + attention, interleaved per head-pair ----
    # V first, then per pair p: Q/K projections for p, then p's attention.
    # This hands ACT its exp work as early as possible instead of letting it
    # idle through the whole QKV phase.
    for si in range(8):  # V in [s, d] orientation (transposed for free)
        pv = ps.tile([P, 512], F32, tag="mm2")
        for kc in range(CT):
            nc.tensor.matmul(
                pv[:, :],
                lhsT=h_all[:, kc, si * 128:(si + 1) * 128],
                rhs=wqkv[:, kc, 1024:1536],
                start=(kc == 0), stop=(kc == CT - 1),
            )
        nc.vector.tensor_add(
            out=vt[:, si, :, 0:D],
            in0=pv[:, :].rearrange("p (h d) -> p h d", h=HEADS),
            in1=vb[:, :].rearrange("p (h d) -> p h d", h=HEADS),
        )
        nc.vector.tensor_copy(
            out=vt[:, si, :, D:M_AV],
            in_=ones64[:, :].rearrange("p (h o) -> p h o", h=HEADS),
        )

    def emit_qk_proj(pc):
        for oi in (pc, pc + 4):  # Q tile then K tile for this pair
            pq = ps.tile([P, S], F32, tag="mm2", name=f"pq{oi}")
            for kc in range(CT):
                for nh in range(2):
                    nc.tensor.matmul(
                        pq[:, nh * 512:(nh + 1) * 512],
                        lhsT=wqkv[:, kc, oi * 128:(oi + 1) * 128],
                        rhs=h_all[:, kc, nh * 512:(nh + 1) * 512],
                        start=(kc == 0), stop=(kc == CT - 1),
                    )
            if oi < 4:  # q: packed pair tile, one eviction
                nc.vector.tensor_scalar(
                    out=qk[:, oi, :], in0=pq[:, :],
                    scalar1=qkvb[:, oi:oi + 1], scalar2=None, op0=OP.add,
                )
            else:  # k: split into per-head padded tiles (partitions preserved)
                hd = 2 * (oi - 4)
                nc.vector.tensor_scalar(
                    out=qk[0:64, 4 + hd, :], in0=pq[0:64, :],
                    scalar1=qkvb[0:64, oi:oi + 1], scalar2=None, op0=OP.add,
                )
                nc.vector.tensor_scalar(
                    out=qk[64:P, 4 + hd + 1, :], in0=pq[64:P, :],
                    scalar1=qkvb[64:P, oi:oi + 1], scalar2=None, op0=OP.add,
                )

    def emit_attention(pc):
        pt = ptp.tile([P, 2, 8, S], BF16, tag="pt", name=f"pt{pc}")
        for bi in range(8):
            for hp in range(2):
                hd = 2 * pc + hp
                pS = ps.tile([P, S], F32, tag="mm2", name=f"pS{pc}_{bi}_{hp}")
                for ah in range(2):
                    nc.tensor.matmul(
                        pS[:, ah * 512:(ah + 1) * 512],
                        lhsT=qk[:, 4 + hd, bi * 128:(bi + 1) * 128],
                        rhs=qk[:, pc, ah * 512:(ah + 1) * 512],
                        start=True, stop=True,
                    )
                nc.scalar.activation(out=pt[:, hp, bi, :], in_=pS[:, :], func=AF.Exp, scale=0.125)
        for hp in range(2):
            hd = 2 * pc + hp
            pav = psav.tile([P, S], F32, tag="av", name=f"pav{pc}_{hp}")
            for bi in range(8):
                for ah in range(2):
                    nc.tensor.matmul(
                        pav[0:M_AV, ah * 512:(ah + 1) * 512],
                        lhsT=vt[:, bi, hd, :],
                        rhs=pt[:, hp, bi, ah * 512:(ah + 1) * 512],
                        start=(bi == 0), stop=(bi == 7),
                    )
            zc = small.tile([P, S], F32, tag="zc", name=f"zc{pc}_{hp}")
            nc.vector.tensor_copy(out=zc[0:8, :], in_=pav[D:D + 8, :])
            zs = small.tile([P, S], F32, tag="zs", name=f"zs{pc}_{hp}")
            nc.vector.reciprocal_approx_fast(out=zs[0:8, :], in_=zc[0:8, :])
            zb = small.tile([P, S], F32, tag="zb", name=f"zb{pc}_{hp}")
            nc.gpsimd.partition_broadcast(zb[0:D, :], zs[0:1, :])
            if hp == 0:
                nc.vector.tensor_mul(out=att[0:D, pc, :], in0=pav[0:D, :], in1=zb[0:D, :])
            else:
                stg = small.tile([P, S], BF16, tag="stg", name=f"stg{pc}")
                nc.vector.tensor_mul(out=stg[0:D, :], in0=pav[0:D, :], in1=zb[0:D, :])
                nc.sync.dma_start(out=att[D:P, pc, :], in_=stg[0:D, :])

    # software-pipelined emission: one pair of Q/K lookahead before each attention
    emit_qk_proj(0)
    emit_qk_proj(1)
    emit_attention(0)
    emit_qk_proj(2)
    emit_attention(1)
    emit_qk_proj(3)
    emit_attention(2)
    emit_attention(3)

    # ---- proj + bias + residual ----
    out_view = out_d[:, :].rearrange("(j p) s -> p j s", p=P)
    for oi in range(CT):
        pp = ps.tile([P, S], F32, tag="mm2")
        for pc in range(4):
            for sh in range(2):
                nc.tensor.matmul(
                    pp[:, sh * 512:(sh + 1) * 512],
                    lhsT=wproj[:, pc, oi * 128:(oi + 1) * 128],
                    rhs=att[:, pc, sh * 512:(sh + 1) * 512],
                    start=(pc == 0), stop=(pc == 3),
                )
        ot = outp.tile([P, S], F32, tag="o")
        nc.vector.tensor_scalar(out=ot, in0=pp[:, :], scalar1=projb[:, oi:oi + 1], scalar2=None, op0=OP.add)
        nc.vector.tensor_add(out=ot, in0=ot, in1=x_all[:, oi, :])
        nc.sync.dma_start(out=out_view[:, oi, :], in_=ot)


_NC_CACHE = None


def _build():
    global _NC_CACHE
    if _NC_CACHE is None:
        from contextlib import ExitStack

        nc = bacc.Bacc()
        with tile.TileContext(nc) as tc:
            with ExitStack() as ctx:
                _emit(nc, tc, ctx)
        nc.finalize()
        _NC_CACHE = nc
    return _NC_CACHE


def _prep_inputs(inputs):
    x = np.ascontiguousarray(np.asarray(inputs["x"], dtype=np.float32))  # [8,512,32,32]
    gn_w = np.asarray(inputs["gn_w"], dtype=np.float32)
    gn_b = np.asarray(inputs["gn_b"], dtype=np.float32)
    qkv_w = np.asarray(inputs["qkv_w"], dtype=np.float32)
    qkv_b = np.asarray(inputs["qkv_b"], dtype=np.float32)
    proj_w = np.asarray(inputs["proj_w"], dtype=np.float32)
    proj_b = np.asarray(inputs["proj_b"], dtype=np.float32)

    wqkv_p = np.ascontiguousarray(
        qkv_w.T.reshape(CT, P, 1536).transpose(1, 0, 2).astype(NPBF16)
    )
    wproj_p = np.ascontiguousarray(
        proj_w.T.reshape(CT, P, 512).transpose(1, 0, 2).astype(NPBF16)
    )
    gnw_p = np.ascontiguousarray(gn_w.reshape(CT, P).T)
    gnb_p = np.ascontiguousarray(gn_b.reshape(CT, P).T)
    qkvb_p = np.ascontiguousarray(qkv_b[:1024].reshape(8, P).T)
    vb_p = np.ascontiguousarray(qkv_b[1024:])
    projb_p = np.ascontiguousarray(proj_b.reshape(CT, P).T)
    sel = np.zeros((P, P), dtype=NPBF16)
    for p in range(P):
        sel[p, p // 16] = 1.0
    selt = np.ascontiguousarray(sel.T)

    shared = {
        "wqkv": wqkv_p, "wproj": wproj_p, "gnw": gnw_p, "gnb": gnb_p,
        "qkvb": qkvb_p, "vb": vb_p, "projb": projb_p, "sel": sel, "selt": selt,
    }
    in_maps = []
    for i in range(N_CORES):
        m = dict(shared)
        m["x"] = np.ascontiguousarray(x[i].reshape(512, S))
        in_maps.append(m)
    return in_maps


def run(inputs, trace=False, tmpdir=None):
    nc = _build()
    in_maps = _prep_inputs(inputs)
    res = run_bass_kernel_spmd(
        nc, in_maps, core_ids=list(range(N_CORES)), trace=trace, tmpdir=tmpdir
    )
    out = np.stack([res.results[i]["out"] for i in range(N_CORES)])
    return out.reshape(N_CORES, 512, 32, 32), res


def kernel(**inputs):
    out, _ = run(inputs, trace=False)
    return out


# revision 18
# speedup vs baseline: 1.3338x; 1.0803x over previous
"""AttentionBlock (GroupNorm + 8-head self-attention + proj + residual) on 8 TRN2 cores.

Sharding: data-parallel over batch (8 batch elements -> 8 cores). Each core runs the
full block for one [512, 32*32] image in a single Bass/Tile kernel.

Per-core pipeline (matmul operands in bf16, fp32 accumulation, fp32 elsewhere):
  GroupNorm:   bn_stats/bn_aggr per channel -> group reduce via matmul with a
               0/1 selection matrix -> rsqrt -> broadcast back via matmul ->
               fused scale+bias apply (DVE).
  QKV:         h @ Wqkv^T. Q,K produced as [d, s] zero-padded to 128 partitions
               per head (so attention matmuls run K=128, full array mode);
               V produced transposed ([s, d]) by swapping matmul operands,
               so attention needs no transposes.
  Attention:   S^T[b,a] = K^T Q per head, P^T = exp(S^T/8) on ACT in [128,1024]
               slabs (softmax max-subtraction skipped: logits are O(1) by
               construction), AV via [v^T | 1x8] augmented weights -> psum rows
               0-63 = unnormalized out, rows 64-71 = Z; normalize with a
               multi-lane reciprocal + GPSIMD partition-broadcast.
  Proj+res:    att @ Wproj^T + proj_b + x.
"""
import sys

sys.path.insert(0, "/opt/trn_rl_repo")

import numpy as np
import ml_dtypes

import concourse.bass as bass
import concourse.bacc as bacc
import concourse.tile as tile
from concourse import mybir
from concourse.bass_utils import run_bass_kernel_spmd

F32 = mybir.dt.float32
BF16 = mybir.dt.bfloat16
AF = mybir.ActivationFunctionType
OP = mybir.AluOpType
NPBF16 = ml_dtypes.bfloat16

P = 128
CT = 4  # channel tiles (512 / 128)
S = 1024  # spatial positions (32*32)
HEADS = 8
D = 64
M_AV = D + 8  # AV stationary cols: 64 v + 8 ones (Z lands on psum rows 64-71)
N_CORES = 8
EPS = 1e-5


def _emit(nc, tc, ctx):
    x_d = nc.dram_tensor("x", [512, S], F32, kind="ExternalInput")
    wqkv_d = nc.dram_tensor("wqkv", [P, CT, 1536], BF16, kind="ExternalInput")
    wproj_d = nc.dram_tensor("wproj", [P, CT, 512], BF16, kind="ExternalInput")
    gnw_d = nc.dram_tensor("gnw", [P, CT], F32, kind="ExternalInput")
    gnb_d = nc.dram_tensor("gnb", [P, CT], F32, kind="ExternalInput")
    qkvb_d = nc.dram_tensor("qkvb", [P, 8], F32, kind="ExternalInput")
    vb_d = nc.dram_tensor("vb", [512], F32, kind="ExternalInput")
    projb_d = nc.dram_tensor("projb", [P, CT], F32, kind="ExternalInput")
    sel_d = nc.dram_tensor("sel", [P, P], BF16, kind="ExternalInput")
    selt_d = nc.dram_tensor("selt", [P, P], BF16, kind="ExternalInput")
    out_d = nc.dram_tensor("out", [512, S], F32, kind="ExternalOutput")

    consts = ctx.enter_context(tc.tile_pool(name="consts", bufs=1))
    big = ctx.enter_context(tc.tile_pool(name="big", bufs=1))
    small = ctx.enter_context(tc.tile_pool(name="small", bufs=2))
    ptp = ctx.enter_context(tc.tile_pool(name="ptp", bufs=2))
    outp = ctx.enter_context(tc.tile_pool(name="outp", bufs=3))
    ps = ctx.enter_context(tc.tile_pool(name="ps", bufs=2, space="PSUM"))
    psav = ctx.enter_context(tc.tile_pool(name="psav", bufs=2, space="PSUM"))

    # ---- input DMAs (x first: GroupNorm needs it immediately) ----
    x_all = big.tile([P, CT, S], F32)
    xv = x_d[:, :].rearrange("(j p) s -> p j s", p=P)
    for j in range(CT):
        nc.sync.dma_start(out=x_all[:, j, :], in_=xv[:, j, :])
    gnw = consts.tile([P, CT], F32)
    nc.sync.dma_start(out=gnw, in_=gnw_d[:, :])
    gnb = consts.tile([P, CT], F32)
    nc.sync.dma_start(out=gnb, in_=gnb_d[:, :])
    sel = consts.tile([P, P], BF16)
    nc.gpsimd.dma_start(out=sel, in_=sel_d[:, :])
    selt = consts.tile([P, P], BF16)
    nc.gpsimd.dma_start(out=selt, in_=selt_d[:, :])
    wqkv = consts.tile([P, CT, 1536], BF16)
    nc.gpsimd.dma_start(out=wqkv, in_=wqkv_d[:, :, :])
    qkvb = consts.tile([P, 8], F32)
    nc.sync.dma_start(out=qkvb, in_=qkvb_d[:, :])
    projb = consts.tile([P, CT], F32)
    nc.sync.dma_start(out=projb, in_=projb_d[:, :])
    vb = consts.tile([P, 512], F32)
    vb_ap = vb_d[:]
    vb_bcast = bass.AP(tensor=vb_ap.tensor, offset=vb_ap.offset, ap=[[0, P], vb_ap.ap[0]])
    nc.gpsimd.dma_start(out=vb, in_=vb_bcast)
    wproj = consts.tile([P, CT, 512], BF16)
    nc.gpsimd.dma_start(out=wproj, in_=wproj_d[:, :, :])

    eps_t = consts.tile([P, 1], F32)
    nc.vector.memset(eps_t, EPS)
    zeros8 = consts.tile([P, 8], F32)
    nc.vector.memset(zeros8, 0.0)
    ones64 = consts.tile([P, 64], BF16)
    nc.vector.memset(ones64, 1.0)

    h_all = big.tile([P, CT, S], BF16)
    # q: 4 packed head-pair tiles (0-3). k: 8 per-head tiles (4-11) zero-padded
    # to 128 partitions -- head h's 64 d-rows sit at partitions (h%2)*64, the
    # other half stays zero so attention matmuls run K=128 in full-array mode.
    qk = big.tile([P, 12, S], BF16)
    nc.vector.memset(qk[:, 4:12, :], 0.0)

    vt = big.tile([P, 8, HEADS, M_AV], BF16)
    att = big.tile([P, CT, S], BF16)

    # ---- GroupNorm statistics ----
    stats = small.tile([P, CT, 2, 6], F32)
    mv = small.tile([P, CT, 2], F32)
    for j in range(CT):
        for sg in range(2):
            nc.vector.bn_stats(out=stats[:, j, sg, :], in_=x_all[:, j, sg * 512:(sg + 1) * 512])
        nc.vector.bn_aggr(out=mv[:, j, :], in_=stats[:, j, :, :])
    means = mv[:, :, 0]
    vars_ = mv[:, :, 1]
    stats2 = small.tile([P, 8], F32)
    nc.vector.tensor_copy(out=stats2[:, 0:4], in_=means)
    nc.vector.tensor_mul(out=stats2[:, 4:8], in0=means, in1=means)
    nc.vector.tensor_add(out=stats2[:, 4:8], in0=stats2[:, 4:8], in1=vars_)
    statsr = small.tile([P, 8], BF16)
    nc.vector.tensor_copy(out=statsr, in_=stats2)

    psum_g = ps.tile([P, 8], F32, tag="mm2")
    nc.tensor.matmul(psum_g[:, :], lhsT=sel[:, :], rhs=statsr[:, :], start=True, stop=True)

    tmv = small.tile([P, 8], F32)
    nc.vector.tensor_scalar_mul(out=tmv[0:8, :], in0=psum_g[0:8, :], scalar1=1.0 / 16.0)
    gm = tmv[0:8, 0:4]
    gm2 = tmv[0:8, 4:8]
    var_t = small.tile([P, 4], F32)
    nc.vector.tensor_mul(out=var_t[0:8, :], in0=gm, in1=gm)
    nc.vector.tensor_sub(out=var_t[0:8, :], in0=gm2, in1=var_t[0:8, :])
    nc.scalar.activation(out=var_t[0:8, :], in_=var_t[0:8, :], func=AF.Sqrt, bias=eps_t[0:8, :], scale=1.0)
    a_t = small.tile([P, 4], F32)
    nc.vector.reciprocal(out=a_t[0:8, :], in_=var_t[0:8, :])
    b_t = small.tile([P, 4], F32)
    nc.vector.tensor_mul(out=b_t[0:8, :], in0=gm, in1=a_t[0:8, :])
    abr = small.tile([P, 8], BF16)
    nc.vector.tensor_copy(out=abr, in_=zeros8)
    nc.vector.tensor_copy(out=abr[0:8, 0:4], in_=a_t[0:8, :])
    nc.vector.tensor_scalar(out=abr[0:8, 4:8], in0=b_t[0:8, :], scalar1=-1.0, scalar2=None, op0=OP.mult)

    # GroupNorm group stats in bf16 would lose ~0.4%; the scale/bias path keeps
    # everything fp32 except the two tiny matmul hops (sel/selt are exact 0/1).
    psum_ab = ps.tile([P, 8], F32, tag="mm2")
    nc.tensor.matmul(psum_ab[:, :], lhsT=selt[:, :], rhs=abr[:, :], start=True, stop=True)

    scb = small.tile([P, CT, 2], F32)
    for j in range(CT):
        nc.vector.tensor_mul(out=scb[:, j, 0:1], in0=psum_ab[:, j:j + 1], in1=gnw[:, j:j + 1])
        nc.vector.tensor_mul(out=scb[:, j, 1:2], in0=psum_ab[:, 4 + j:5 + j], in1=gnw[:, j:j + 1])
        nc.vector.tensor_add(out=scb[:, j, 1:2], in0=scb[:, j, 1:2], in1=gnb[:, j:j + 1])
        nc.vector.tensor_scalar(
            out=h_all[:, j, :], in0=x_all[:, j, :],
            scalar1=scb[:, j, 0:1], scalar2=scb[:, j, 1:2],
            op0=OP.mult, op1=OP.add,
        )

    # ---- QKV ----
    for oi in range(8):  # Q (0-3), K (4-7) in [d, s]; o-tile = head pair
        pq = ps.tile([P, S], F32, tag="mm2")
        for kc in range(CT):
            for nh in range(2):
                nc.tensor.matmul(
                    pq[:, nh * 512:(nh + 1) * 512],
                    lhsT=wqkv[:, kc, oi * 128:(oi + 1) * 128],
                    rhs=h_all[:, kc, nh * 512:(nh + 1) * 512],
                    start=(kc == 0), stop=(kc == CT - 1),
                )
        if oi < 4:  # q: packed pair tile, one eviction
            nc.vector.tensor_scalar(
                out=qk[:, oi, :], in0=pq[:, :],
                scalar1=qkvb[:, oi:oi + 1], scalar2=None, op0=OP.add,
            )
        else:  # k: split into per-head padded tiles (partitions preserved)
            hd = 2 * (oi - 4)
            nc.vector.tensor_scalar(
                out=qk[0:64, 4 + hd, :], in0=pq[0:64, :],
                scalar1=qkvb[0:64, oi:oi + 1], scalar2=None, op0=OP.add,
            )
            nc.vector.tensor_scalar(
                out=qk[64:P, 4 + hd + 1, :], in0=pq[64:P, :],
                scalar1=qkvb[64:P, oi:oi + 1], scalar2=None, op0=OP.add,
            )
    for si in range(8):  # V in [s, d] orientation (transposed for free)
        pv = ps.tile([P, 512], F32, tag="mm2")
        for kc in range(CT):
            nc.tensor.matmul(
                pv[:, :],
                lhsT=h_all[:, kc, si * 128:(si + 1) * 128],
                rhs=wqkv[:, kc, 1024:1536],
                start=(kc == 0), stop=(kc == CT - 1),
            )
        nc.vector.tensor_add(
            out=vt[:, si, :, 0:D],
            in0=pv[:, :].rearrange("p (h d) -> p h d", h=HEADS),
            in1=vb[:, :].rearrange("p (h d) -> p h d", h=HEADS),
        )
        nc.vector.tensor_copy(
            out=vt[:, si, :, D:M_AV],
            in_=ones64[:, :].rearrange("p (h o) -> p h o", h=HEADS),
        )

    # ---- attention ----
    for pc in range(4):  # head pairs
        pt = ptp.tile([P, 2, 8, S], BF16, tag="pt")
        for bi in range(8):
            for hp in range(2):
                hd = 2 * pc + hp
                pS = ps.tile([P, S], F32, tag="mm2")
                for ah in range(2):
                    nc.tensor.matmul(
                        pS[:, ah * 512:(ah + 1) * 512],
                        lhsT=qk[:, 4 + hd, bi * 128:(bi + 1) * 128],
                        rhs=qk[:, pc, ah * 512:(ah + 1) * 512],
                        start=True, stop=True,
                    )
                nc.scalar.activation(out=pt[:, hp, bi, :], in_=pS[:, :], func=AF.Exp, scale=0.125)
        for hp in range(2):
            hd = 2 * pc + hp
            pav = psav.tile([P, S], F32, tag="av")
            for bi in range(8):
                for ah in range(2):
                    nc.tensor.matmul(
                        pav[0:M_AV, ah * 512:(ah + 1) * 512],
                        lhsT=vt[:, bi, hd, :],
                        rhs=pt[:, hp, bi, ah * 512:(ah + 1) * 512],
                        start=(bi == 0), stop=(bi == 7),
                    )
            zc = small.tile([P, S], F32, tag="zc")
            nc.vector.tensor_copy(out=zc[0:8, :], in_=pav[D:D + 8, :])
            zs = small.tile([P, S], F32, tag="zs")
            nc.vector.reciprocal_approx_fast(out=zs[0:8, :], in_=zc[0:8, :])
            zb = small.tile([P, S], F32, tag="zb")
            nc.gpsimd.partition_broadcast(zb[0:D, :], zs[0:1, :])
            if hp == 0:
                nc.vector.tensor_mul(out=att[0:D, pc, :], in0=pav[0:D, :], in1=zb[0:D, :])
            else:
                stg = small.tile([P, S], BF16, tag="stg")
                nc.vector.tensor_mul(out=stg[0:D, :], in0=pav[0:D, :], in1=zb[0:D, :])
                nc.sync.dma_start(out=att[D:P, pc, :], in_=stg[0:D, :])

    # ---- proj + bias + residual ----
    out_view = out_d[:, :].rearrange("(j p) s -> p j s", p=P)
    for oi in range(CT):
        pp = ps.tile([P, S], F32, tag="mm2")
        for pc in range(4):
            for sh in range(2):
                nc.tensor.matmul(
                    pp[:, sh * 512:(sh + 1) * 512],
                    lhsT=wproj[:, pc, oi * 128:(oi + 1) * 128],
                    rhs=att[:, pc, sh * 512:(sh + 1) * 512],
                    start=(pc == 0), stop=(pc == 3),
                )
        ot = outp.tile([P, S], F32, tag="o")
        nc.vector.tensor_scalar(out=ot, in0=pp[:, :], scalar1=projb[:, oi:oi + 1], scalar2=None, op0=OP.add)
        nc.vector.tensor_add(out=ot, in0=ot, in1=x_all[:, oi, :])
        nc.sync.dma_start(out=out_view[:, oi, :], in_=ot)


_NC_CACHE = None


def _build():
    global _NC_CACHE
    if _NC_CACHE is None:
        from contextlib import ExitStack

        nc = bacc.Bacc()
        with tile.TileContext(nc) as tc:
            with ExitStack() as ctx:
                _emit(nc, tc, ctx)
        nc.finalize()
        _NC_CACHE = nc
    return _NC_CACHE


def _prep_inputs(inputs):
    x = np.ascontiguousarray(np.asarray(inputs["x"], dtype=np.float32))  # [8,512,32,32]
    gn_w = np.asarray(inputs["gn_w"], dtype=np.float32)
    gn_b = np.asarray(inputs["gn_b"], dtype=np.float32)
    qkv_w = np.asarray(inputs["qkv_w"], dtype=np.float32)
    qkv_b = np.asarray(inputs["qkv_b"], dtype=np.float32)
    proj_w = np.asarray(inputs["proj_w"], dtype=np.float32)
    proj_b = np.asarray(inputs["proj_b"], dtype=np.float32)

    wqkv_p = np.ascontiguousarray(
        qkv_w.T.reshape(CT, P, 1536).transpose(1, 0, 2).astype(NPBF16)
    )
    wproj_p = np.ascontiguousarray(
        proj_w.T.reshape(CT, P, 512).transpose(1, 0, 2).astype(NPBF16)
    )
    gnw_p = np.ascontiguousarray(gn_w.reshape(CT, P).T)
    gnb_p = np.ascontiguousarray(gn_b.reshape(CT, P).T)
    qkvb_p = np.ascontiguousarray(qkv_b[:1024].reshape(8, P).T)
    vb_p = np.ascontiguousarray(qkv_b[1024:])
    projb_p = np.ascontiguousarray(proj_b.reshape(CT, P).T)
    sel = np.zeros((P, P), dtype=NPBF16)
    for p in range(P):
        sel[p, p // 16] = 1.0
    selt = np.ascontiguousarray(sel.T)

    shared = {
        "wqkv": wqkv_p, "wproj": wproj_p, "gnw": gnw_p, "gnb": gnb_p,
        "qkvb": qkvb_p, "vb": vb_p, "projb": projb_p, "sel": sel, "selt": selt,
    }
    in_maps = []
    for i in range(N_CORES):
        m = dict(shared)
        m["x"] = np.ascontiguousarray(x[i].reshape(512, S))
        in_maps.append(m)
    return in_maps


def run(inputs, trace=False, tmpdir=None):
    nc = _build()
    in_maps = _prep_inputs(inputs)
    res = run_bass_kernel_spmd(
        nc, in_maps, core_ids=list(range(N_CORES)), trace=trace, tmpdir=tmpdir
    )
    out = np.stack([res.results[i]["out"] for i in range(N_CORES)])
    return out.reshape(N_CORES, 512, 32, 32), res


def kernel(**inputs):
    out, _ = run(inputs, trace=False)
    return out


# revision 19
# speedup vs baseline: 1.3737x; 1.0299x over previous
"""AttentionBlock (GroupNorm + 8-head self-attention + proj + residual) on 8 TRN2 cores.

Sharding: data-parallel over batch (8 batch elements -> 8 cores). Each core runs the
full block for one [512, 32*32] image in a single Bass/Tile kernel.

Per-core pipeline (matmul operands in bf16, fp32 accumulation, fp32 elsewhere):
  GroupNorm:   bn_stats/bn_aggr per channel -> group reduce via matmul with a
               0/1 selection matrix -> rsqrt -> broadcast back via matmul ->
               fused scale+bias apply (DVE).
  QKV:         h @ Wqkv^T. Q,K produced as [d, s] zero-padded to 128 partitions
               per head (so attention matmuls run K=128, full array mode);
               V produced transposed ([s, d]) by swapping matmul operands,
               so attention needs no transposes.
  Attention:   S^T[b,a] = K^T Q per head, P^T = exp(S^T/8) on ACT in [128,1024]
               slabs (softmax max-subtraction skipped: logits are O(1) by
               construction), AV via [v^T | 1x8] augmented weights -> psum rows
               0-63 = unnormalized out, rows 64-71 = Z; normalize with a
               multi-lane reciprocal + GPSIMD partition-broadcast.
  Proj+res:    att @ Wproj^T + proj_b + x.
"""
import sys

sys.path.insert(0, "/opt/trn_rl_repo")

import numpy as np
import ml_dtypes

import concourse.bass as bass
import concourse.bacc as bacc
import concourse.tile as tile
from concourse import mybir
from concourse.bass_utils import run_bass_kernel_spmd

F32 = mybir.dt.float32
BF16 = mybir.dt.bfloat16
AF = mybir.ActivationFunctionType
OP = mybir.AluOpType
NPBF16 = ml_dtypes.bfloat16

P = 128
CT = 4  # channel tiles (512 / 128)
S = 1024  # spatial positions (32*32)
HEADS = 8
D = 64
M_AV = D + 8  # AV stationary cols: 64 v + 8 ones (Z lands on psum rows 64-71)
N_CORES = 8
EPS = 1e-5


def _emit(nc, tc, ctx):
    x_d = nc.dram_tensor("x", [512, S], F32, kind="ExternalInput")
    wqkv_d = nc.dram_tensor("wqkv", [P, CT, 1536], BF16, kind="ExternalInput")
    wproj_d = nc.dram_tensor("wproj", [P, CT, 512], BF16, kind="ExternalInput")
    gnw_d = nc.dram_tensor("gnw", [P, CT], F32, kind="ExternalInput")
    gnb_d = nc.dram_tensor("gnb", [P, CT], F32, kind="ExternalInput")
    qkvb_d = nc.dram_tensor("qkvb", [P, 8], F32, kind="ExternalInput")
    vb_d = nc.dram_tensor("vb", [512], F32, kind="ExternalInput")
    projb_d = nc.dram_tensor("projb", [P, CT], F32, kind="ExternalInput")
    sel_d = nc.dram_tensor("sel", [P, P], BF16, kind="ExternalInput")
    selt_d = nc.dram_tensor("selt", [P, P], BF16, kind="ExternalInput")
    out_d = nc.dram_tensor("out", [512, S], F32, kind="ExternalOutput")

    consts = ctx.enter_context(tc.tile_pool(name="consts", bufs=1))
    big = ctx.enter_context(tc.tile_pool(name="big", bufs=1))
    small = ctx.enter_context(tc.tile_pool(name="small", bufs=2))
    ptp = ctx.enter_context(tc.tile_pool(name="ptp", bufs=2))
    outp = ctx.enter_context(tc.tile_pool(name="outp", bufs=3))
    ps = ctx.enter_context(tc.tile_pool(name="ps", bufs=2, space="PSUM"))
    psav = ctx.enter_context(tc.tile_pool(name="psav", bufs=2, space="PSUM"))

    # ---- input DMAs (x first: GroupNorm needs it immediately) ----
    x_all = big.tile([P, CT, S], F32)
    xv = x_d[:, :].rearrange("(j p) s -> p j s", p=P)
    for j in range(CT):
        nc.sync.dma_start(out=x_all[:, j, :], in_=xv[:, j, :])
    gnw = consts.tile([P, CT], F32)
    nc.sync.dma_start(out=gnw, in_=gnw_d[:, :])
    gnb = consts.tile([P, CT], F32)
    nc.sync.dma_start(out=gnb, in_=gnb_d[:, :])
    sel = consts.tile([P, P], BF16)
    nc.sync.dma_start(out=sel, in_=sel_d[:, :])
    selt = consts.tile([P, P], BF16)
    nc.sync.dma_start(out=selt, in_=selt_d[:, :])
    wqkv = consts.tile([P, CT, 1536], BF16)
    nc.sync.dma_start(out=wqkv, in_=wqkv_d[:, :, :])
    qkvb = consts.tile([P, 8], F32)
    nc.sync.dma_start(out=qkvb, in_=qkvb_d[:, :])
    projb = consts.tile([P, CT], F32)
    nc.sync.dma_start(out=projb, in_=projb_d[:, :])
    vb = consts.tile([P, 512], F32)
    vb_ap = vb_d[:]
    vb_bcast = bass.AP(tensor=vb_ap.tensor, offset=vb_ap.offset, ap=[[0, P], vb_ap.ap[0]])
    nc.gpsimd.dma_start(out=vb, in_=vb_bcast)
    wproj = consts.tile([P, CT, 512], BF16)
    nc.sync.dma_start(out=wproj, in_=wproj_d[:, :, :])

    eps_t = consts.tile([P, 1], F32)
    nc.vector.memset(eps_t, EPS)
    zeros8 = consts.tile([P, 8], F32)
    nc.vector.memset(zeros8, 0.0)
    ones64 = consts.tile([P, 64], BF16)
    nc.vector.memset(ones64, 1.0)

    h_all = big.tile([P, CT, S], BF16)
    # q: 4 packed head-pair tiles (0-3). k: 8 per-head tiles (4-11) zero-padded
    # to 128 partitions -- head h's 64 d-rows sit at partitions (h%2)*64, the
    # other half stays zero so attention matmuls run K=128 in full-array mode.
    qk = big.tile([P, 12, S], BF16)
    nc.vector.memset(qk[:, 4:12, :], 0.0)

    vt = big.tile([P, 8, HEADS, M_AV], BF16)
    atts = [big.tile([P, S], BF16, name=f"att{i}") for i in range(4)]

    # ---- GroupNorm statistics ----
    stats = small.tile([P, CT, 2, 6], F32)
    mv = small.tile([P, CT, 2], F32)
    for j in range(CT):
        for sg in range(2):
            nc.vector.bn_stats(out=stats[:, j, sg, :], in_=x_all[:, j, sg * 512:(sg + 1) * 512])
        nc.vector.bn_aggr(out=mv[:, j, :], in_=stats[:, j, :, :])
    means = mv[:, :, 0]
    vars_ = mv[:, :, 1]
    stats2 = small.tile([P, 8], F32)
    nc.vector.tensor_copy(out=stats2[:, 0:4], in_=means)
    nc.vector.tensor_mul(out=stats2[:, 4:8], in0=means, in1=means)
    nc.vector.tensor_add(out=stats2[:, 4:8], in0=stats2[:, 4:8], in1=vars_)
    statsr = small.tile([P, 8], BF16)
    nc.vector.tensor_copy(out=statsr, in_=stats2)

    psum_g = ps.tile([P, 8], F32, tag="mm2")
    nc.tensor.matmul(psum_g[:, :], lhsT=sel[:, :], rhs=statsr[:, :], start=True, stop=True)

    tmv = small.tile([P, 8], F32)
    nc.vector.tensor_scalar_mul(out=tmv[0:8, :], in0=psum_g[0:8, :], scalar1=1.0 / 16.0)
    gm = tmv[0:8, 0:4]
    gm2 = tmv[0:8, 4:8]
    var_t = small.tile([P, 4], F32)
    nc.vector.tensor_mul(out=var_t[0:8, :], in0=gm, in1=gm)
    nc.vector.tensor_sub(out=var_t[0:8, :], in0=gm2, in1=var_t[0:8, :])
    nc.scalar.activation(out=var_t[0:8, :], in_=var_t[0:8, :], func=AF.Sqrt, bias=eps_t[0:8, :], scale=1.0)
    a_t = small.tile([P, 4], F32)
    nc.vector.reciprocal(out=a_t[0:8, :], in_=var_t[0:8, :])
    b_t = small.tile([P, 4], F32)
    nc.vector.tensor_mul(out=b_t[0:8, :], in0=gm, in1=a_t[0:8, :])
    abr = small.tile([P, 8], BF16)
    nc.vector.tensor_copy(out=abr, in_=zeros8)
    nc.vector.tensor_copy(out=abr[0:8, 0:4], in_=a_t[0:8, :])
    nc.vector.tensor_scalar(out=abr[0:8, 4:8], in0=b_t[0:8, :], scalar1=-1.0, scalar2=None, op0=OP.mult)

    # GroupNorm group stats in bf16 would lose ~0.4%; the scale/bias path keeps
    # everything fp32 except the two tiny matmul hops (sel/selt are exact 0/1).
    psum_ab = ps.tile([P, 8], F32, tag="mm2")
    nc.tensor.matmul(psum_ab[:, :], lhsT=selt[:, :], rhs=abr[:, :], start=True, stop=True)

    scb = small.tile([P, CT, 2], F32)
    for j in range(CT):
        nc.vector.tensor_mul(out=scb[:, j, 0:1], in0=psum_ab[:, j:j + 1], in1=gnw[:, j:j + 1])
        nc.vector.tensor_mul(out=scb[:, j, 1:2], in0=psum_ab[:, 4 + j:5 + j], in1=gnw[:, j:j + 1])
        nc.vector.tensor_add(out=scb[:, j, 1:2], in0=scb[:, j, 1:2], in1=gnb[:, j:j + 1])
        nc.vector.tensor_scalar(
            out=h_all[:, j, :], in0=x_all[:, j, :],
            scalar1=scb[:, j, 0:1], scalar2=scb[:, j, 1:2],
            op0=OP.mult, op1=OP.add,
        )

    # ---- QKV ----
    for oi in range(8):  # Q (0-3), K (4-7) in [d, s]; o-tile = head pair
        pq = ps.tile([P, S], F32, tag="mm2")
        for kc in range(CT):
            for nh in range(2):
                nc.tensor.matmul(
                    pq[:, nh * 512:(nh + 1) * 512],
                    lhsT=wqkv[:, kc, oi * 128:(oi + 1) * 128],
                    rhs=h_all[:, kc, nh * 512:(nh + 1) * 512],
                    start=(kc == 0), stop=(kc == CT - 1),
                )
        if oi < 4:  # q: packed pair tile, one eviction
            nc.vector.tensor_scalar(
                out=qk[:, oi, :], in0=pq[:, :],
                scalar1=qkvb[:, oi:oi + 1], scalar2=None, op0=OP.add,
            )
        else:  # k: split into per-head padded tiles (partitions preserved)
            hd = 2 * (oi - 4)
            nc.vector.tensor_scalar(
                out=qk[0:64, 4 + hd, :], in0=pq[0:64, :],
                scalar1=qkvb[0:64, oi:oi + 1], scalar2=None, op0=OP.add,
            )
            nc.vector.tensor_scalar(
                out=qk[64:P, 4 + hd + 1, :], in0=pq[64:P, :],
                scalar1=qkvb[64:P, oi:oi + 1], scalar2=None, op0=OP.add,
            )
    for si in range(8):  # V in [s, d] orientation (transposed for free)
        pv = ps.tile([P, 512], F32, tag="mm2")
        for kc in range(CT):
            nc.tensor.matmul(
                pv[:, :],
                lhsT=h_all[:, kc, si * 128:(si + 1) * 128],
                rhs=wqkv[:, kc, 1024:1536],
                start=(kc == 0), stop=(kc == CT - 1),
            )
        nc.vector.tensor_add(
            out=vt[:, si, :, 0:D],
            in0=pv[:, :].rearrange("p (h d) -> p h d", h=HEADS),
            in1=vb[:, :].rearrange("p (h d) -> p h d", h=HEADS),
        )
        nc.vector.tensor_copy(
            out=vt[:, si, :, D:M_AV],
            in_=ones64[:, :].rearrange("p (h o) -> p h o", h=HEADS),
        )

    # ---- attention ----
    for pc in range(4):  # head pairs
        pt = ptp.tile([P, 2, 8, S], BF16, tag="pt")
        for bi in range(8):
            for hp in range(2):
                hd = 2 * pc + hp
                pS = ps.tile([P, S], F32, tag="mm2")
                for ah in range(2):
                    nc.tensor.matmul(
                        pS[:, ah * 512:(ah + 1) * 512],
                        lhsT=qk[:, 4 + hd, bi * 128:(bi + 1) * 128],
                        rhs=qk[:, pc, ah * 512:(ah + 1) * 512],
                        start=True, stop=True,
                    )
                nc.scalar.activation(out=pt[:, hp, bi, :], in_=pS[:, :], func=AF.Exp, scale=0.125)
        for hp in range(2):
            hd = 2 * pc + hp
            pav = psav.tile([P, S], F32, tag="av")
            for bi in range(8):
                for ah in range(2):
                    nc.tensor.matmul(
                        pav[0:M_AV, ah * 512:(ah + 1) * 512],
                        lhsT=vt[:, bi, hd, :],
                        rhs=pt[:, hp, bi, ah * 512:(ah + 1) * 512],
                        start=(bi == 0), stop=(bi == 7),
                    )
            zc = small.tile([P, S], F32, tag="zc")
            nc.vector.tensor_copy(out=zc[0:8, :], in_=pav[D:D + 8, :])
            zs = small.tile([P, S], F32, tag="zs")
            nc.vector.reciprocal_approx_fast(out=zs[0:8, :], in_=zc[0:8, :])
            zb = small.tile([P, S], F32, tag="zb")
            nc.gpsimd.partition_broadcast(zb[0:D, :], zs[0:1, :])
            if hp == 0:
                nc.vector.tensor_mul(out=atts[pc][0:D, :], in0=pav[0:D, :], in1=zb[0:D, :])
            else:
                stg = small.tile([P, S], BF16, tag="stg")
                nc.vector.tensor_mul(out=stg[0:D, :], in0=pav[0:D, :], in1=zb[0:D, :])
                nc.sync.dma_start(out=atts[pc][D:P, :], in_=stg[0:D, :])

    # ---- proj + bias + residual ----
    out_view = out_d[:, :].rearrange("(j p) s -> p j s", p=P)
    for oi in range(CT):
        pp = ps.tile([P, S], F32, tag="mm2")
        for pc in range(4):
            for sh in range(2):
                nc.tensor.matmul(
                    pp[:, sh * 512:(sh + 1) * 512],
                    lhsT=wproj[:, pc, oi * 128:(oi + 1) * 128],
                    rhs=atts[pc][:, sh * 512:(sh + 1) * 512],
                    start=(pc == 0), stop=(pc == 3),
                )
        ot = outp.tile([P, S], F32, tag="o")
        nc.vector.tensor_scalar(out=ot, in0=pp[:, :], scalar1=projb[:, oi:oi + 1], scalar2=None, op0=OP.add)
        nc.vector.tensor_add(out=ot, in0=ot, in1=x_all[:, oi, :])
        nc.sync.dma_start(out=out_view[:, oi, :], in_=ot)


_NC_CACHE = None


def _build():
    global _NC_CACHE
    if _NC_CACHE is None:
        from contextlib import ExitStack

        nc = bacc.Bacc()
        with tile.TileContext(nc) as tc:
            with ExitStack() as ctx:
                _emit(nc, tc, ctx)
        nc.finalize()
        _NC_CACHE = nc
    return _NC_CACHE


def _prep_inputs(inputs):
    x = np.ascontiguousarray(np.asarray(inputs["x"], dtype=np.float32))  # [8,512,32,32]
    gn_w = np.asarray(inputs["gn_w"], dtype=np.float32)
    gn_b = np.asarray(inputs["gn_b"], dtype=np.float32)
    qkv_w = np.asarray(inputs["qkv_w"], dtype=np.float32)
    qkv_b = np.asarray(inputs["qkv_b"], dtype=np.float32)
    proj_w = np.asarray(inputs["proj_w"], dtype=np.float32)
    proj_b = np.asarray(inputs["proj_b"], dtype=np.float32)

    wqkv_p = np.ascontiguousarray(
        qkv_w.T.reshape(CT, P, 1536).transpose(1, 0, 2).astype(NPBF16)
    )
    wproj_p = np.ascontiguousarray(
        proj_w.T.reshape(CT, P, 512).transpose(1, 0, 2).astype(NPBF16)
    )
    gnw_p = np.ascontiguousarray(gn_w.reshape(CT, P).T)
    gnb_p = np.ascontiguousarray(gn_b.reshape(CT, P).T)
    qkvb_p = np.ascontiguousarray(qkv_b[:1024].reshape(8, P).T)
    vb_p = np.ascontiguousarray(qkv_b[1024:])
    projb_p = np.ascontiguousarray(proj_b.reshape(CT, P).T)
    sel = np.zeros((P, P), dtype=NPBF16)
    for p in range(P):
        sel[p, p // 16] = 1.0
    selt = np.ascontiguousarray(sel.T)

    shared = {
        "wqkv": wqkv_p, "wproj": wproj_p, "gnw": gnw_p, "gnb": gnb_p,
        "qkvb": qkvb_p, "vb": vb_p, "projb": projb_p, "sel": sel, "selt": selt,
    }
    in_maps = []
    for i in range(N_CORES):
        m = dict(shared)
        m["x"] = np.ascontiguousarray(x[i].reshape(512, S))
        in_maps.append(m)
    return in_maps


def run(inputs, trace=False, tmpdir=None):
    nc = _build()
    in_maps = _prep_inputs(inputs)
    res = run_bass_kernel_spmd(
        nc, in_maps, core_ids=list(range(N_CORES)), trace=trace, tmpdir=tmpdir
    )
    out = np.stack([res.results[i]["out"] for i in range(N_CORES)])
    return out.reshape(N_CORES, 512, 32, 32), res


def kernel(**inputs):
    out, _ = run(inputs, trace=False)
    return out
